# revision 34
# baseline (speedup 1.0000x reference)
"""Multi-head attention (QKV proj + per-head RMSNorm + softmax attention +
output proj) for Trainium2, distributed over 8 NeuronCores.

Sharding: batch (2) x head-groups (4 heads per core).  Each core computes, for
its batch element and its 4 heads: Q^T/K^T projections (transposed layout
[d, T], from a host-pretransposed X^T), per-head RMSNorm, S^T = K^T.T @ Q^T
scores in [key, query] layout, exp with no max subtraction, O^T accumulation
via a fused [V|1] matmul, normalization, and a partial output projection
Out^T = Wo_slice.T @ O^T.  The host sums the 4 partial outputs per batch and
transposes back.

Optimization notes (v2):
- K is never normalized: rstd_k/8 rides into the score exp as a per-partition
  activation scale (scores are [key, query] so rstd_k is per-partition).
  The 1/8 fold happens in log-space via the exp bias of the rstd computation.
- ScalarE (ACT) does exp only during the attention phase; all PSUM evictions
  run on DVE.  Softmax denominators use one DVE reciprocal_approx_fast.
- Attention inner loop is software-pipelined: scores(kt+1) is emitted before
  AV(kt) so the PE never waits on ACT's exp.
- Startup DMAs are split across sync+scalar queues and x is loaded in 16
  chunks so the first projection matmuls start within ~5us.
"""

import os
import sys

for _p in ("/opt/trn_rl_repo",):
    if _p not in sys.path:
        sys.path.insert(0, _p)

import numpy as np

B = 2
T = 2048
D = 1024
H = 16
HD = 64
HPC = 4          # heads per core
N_CORES = 8
EPS = 1e-5

_COMPILED = None
LAST_EXEC_NS = None


def _install_ntff_shim():
    """antenv.axon_hooks is missing in this image; provide it so that
    BASS_TRACE=1 profiling works (mirrors trn_boot's ctypes hook)."""
    import contextlib
    import ctypes
    import types

    if "antenv.axon_hooks" in sys.modules:
        return
    so_path = "/opt/axon/libaxon_pjrt.so"
    if not os.path.exists(so_path):
        return
    lib = ctypes.CDLL(so_path)
    if not hasattr(lib, "axon_start_nrt_profile"):
        return
    lib.axon_start_nrt_profile.argtypes = [ctypes.POINTER(ctypes.c_int64), ctypes.c_size_t]
    lib.axon_start_nrt_profile.restype = ctypes.c_int64
    lib.axon_stop_nrt_profile.argtypes = [ctypes.c_char_p]
    lib.axon_stop_nrt_profile.restype = ctypes.c_int64

    @contextlib.contextmanager
    def _hook(output_dir, device_ids):
        import jax

        jax.devices()
        if device_ids:
            ids = (ctypes.c_int64 * len(device_ids))(*device_ids)
            rc = lib.axon_start_nrt_profile(ids, len(device_ids))
        else:
            rc = lib.axon_start_nrt_profile(None, 0)
        if rc != 0:
            raise RuntimeError(f"axon_start_nrt_profile rc={rc}")
        try:
            yield
        finally:
            n = lib.axon_stop_nrt_profile(str(output_dir).encode())
            print(f"profile: {n} file(s) written to {output_dir}", file=sys.stderr)

    mod = types.ModuleType("antenv.axon_hooks")
    mod._hook = _hook
    mod.get_axon_ntff_profile_hook = lambda: mod._hook
    mod.set_axon_ntff_profile_hook = lambda h: setattr(mod, "_hook", h)
    sys.modules["antenv.axon_hooks"] = mod
    try:
        import antenv

        antenv.axon_hooks = mod
    except ImportError:
        pass


def _build():
    from collections import deque

    import concourse.bass as bass
    import concourse.tile as tile
    from concourse import bacc, mybir

    F32 = mybir.dt.float32
    F32R = mybir.dt.float32r
    BF16 = mybir.dt.bfloat16
    HOT = BF16
    Exp = mybir.ActivationFunctionType.Exp
    Log = mybir.ActivationFunctionType.Ln if hasattr(
        mybir.ActivationFunctionType, "Ln") else mybir.ActivationFunctionType.Log

    TT = T // 128            # 16 t-tiles
    CT = D // 128            # 8 contraction tiles over model dim
    QH = T // 1024           # 2 query halves
    NPAIR = HPC // 2         # 2 head pairs per core

    nc = bacc.Bacc("TRN2", target_bir_lowering=False, debug=False, num_devices=N_CORES)

    HIN = BF16
    xbT = nc.dram_tensor("xbT", (D, T), HIN, kind="ExternalInput").ap()
    wq_s = nc.dram_tensor("wq_s", (D, HPC * HD), HIN, kind="ExternalInput").ap()
    wk_s = nc.dram_tensor("wk_s", (D, HPC * HD), HIN, kind="ExternalInput").ap()
    wv_s = nc.dram_tensor("wv_s", (D, HPC * HD), HIN, kind="ExternalInput").ap()
    wo_s = nc.dram_tensor("wo_s", (HPC * HD, D), HIN, kind="ExternalInput").ap()
    ident_d = nc.dram_tensor("ident", (128, 128), HIN, kind="ExternalInput").ap()
    bd2_d = nc.dram_tensor("bd2", (128, 2), HIN, kind="ExternalInput").ap()
    sel_d = [nc.dram_tensor(f"sel{p}", (128, 128), F32, kind="ExternalInput").ap()
             for p in range(NPAIR)]
    outT = nc.dram_tensor("outT", (D, T), HIN, kind="ExternalOutput").ap()

    with tile.TileContext(nc) as tc:
        from contextlib import ExitStack

        with ExitStack() as top:
            # ---- pools (single merged scope) ---------------------------------
            consts = top.enter_context(tc.tile_pool(name="consts", bufs=1))
            qkpool = top.enter_context(tc.tile_pool(name="qk", bufs=1))
            vppool = top.enter_context(tc.tile_pool(name="vp", bufs=1))
            drp = top.enter_context(tc.tile_pool(name="drs", bufs=1, space="DRAM"))
            # PSUM: 2x1-bank (pj/vb/rb/ops halves) + 2x2-bank (scores + ss)
            # + 2x1-bank (o halves) = 8 banks
            ps_big = top.enter_context(tc.tile_pool(name="psbig", bufs=2, space="PSUM"))
            ps_s = top.enter_context(tc.tile_pool(name="pss", bufs=2, space="PSUM"))
            ps_o = top.enter_context(tc.tile_pool(name="pso", bufs=2, space="PSUM"))
            xtp = top.enter_context(tc.tile_pool(name="xT", bufs=1))
            wpool = top.enter_context(tc.tile_pool(name="w", bufs=1))
            qtp = top.enter_context(tc.tile_pool(name="qt", bufs=4))
            vtp = top.enter_context(tc.tile_pool(name="vt", bufs=2))
            q2p = top.enter_context(tc.tile_pool(name="q2", bufs=2))
            nsml = top.enter_context(tc.tile_pool(name="nsml", bufs=1))
            ppool = top.enter_context(tc.tile_pool(name="p", bufs=3))
            dntp = top.enter_context(tc.tile_pool(name="dnt", bufs=2))
            dnp = top.enter_context(tc.tile_pool(name="dn", bufs=1))
            ohpool = top.enter_context(tc.tile_pool(name="ohp", bufs=1))
            outsbp = top.enter_context(tc.tile_pool(name="outsb", bufs=2))
            wop = top.enter_context(tc.tile_pool(name="wo", bufs=1))

            # ---- consts ------------------------------------------------------
            ident = consts.tile([128, 128], HOT, tag="ident")
            nc.sync.dma_start(out=ident[:], in_=ident_d.bitcast(HOT))
            epsc = consts.tile([2, 1], F32, tag="epsc")
            nc.vector.memset(epsc[:], EPS)
            bd2 = consts.tile([128, 2], HOT, tag="bd2")
            nc.sync.dma_start(out=bd2[:], in_=bd2_d)
            sel = []
            for p in range(NPAIR):
                se = consts.tile([128, 128], F32, tag=f"sel{p}", name=f"sel{p}")
                nc.sync.dma_start(out=se[:], in_=sel_d[p])
                sel.append(se)

            # ---- persistent data tiles --------------------------------------
            qhat = [qkpool.tile([128, T], HOT, tag=f"qh{h}", name=f"qhat{h}")
                    for h in range(HPC)]
            khat = [qkpool.tile([128, T], HOT, tag=f"kh{h}", name=f"khat{h}")
                    for h in range(HPC)]
            for h in range(HPC):
                nc.gpsimd.memset(qhat[h][:], 0.0)
                nc.gpsimd.memset(khat[h][:], 0.0)
            vp = [vppool.tile([128, TT, 2, 65], HOT, tag=f"vs{p}", name=f"vst{p}")
                  for p in range(NPAIR)]
            for p in range(NPAIR):
                nc.vector.memset(vp[p][:, :, :, 64:65], 1.0)

            # ---- input DMAs (few big transfers; each stripes over all
            # 16 DMA engines) ---------------------------------------------
            xTt = xtp.tile([128, CT, T], HOT, tag="xT", name="xTt")
            wts_q = wpool.tile([128, CT, HPC * HD], HOT, tag="wq", name="wtq")
            wts_k = wpool.tile([128, CT, HPC * HD], HOT, tag="wk", name="wtk")
            wts_v = wpool.tile([128, CT, HPC * HD], HOT, tag="wv", name="wtv")
            wo_sb = wop.tile([128, NPAIR, D], HOT, tag="wo", name="wot")
            nc.scalar.dma_start(
                out=wts_q[:, 0:4, :],
                in_=wq_s[0:512, :].rearrange("(c p) n -> p c n", c=4))
            nc.sync.dma_start(
                out=xTt[:, 0:2, :],
                in_=xbT[0:256, :].rearrange("(c p) t -> p c t", c=2))
            nc.scalar.dma_start(
                out=wts_q[:, 4:8, :],
                in_=wq_s[512:1024, :].rearrange("(c p) n -> p c n", c=4))
            for g in range(1, 4):
                nc.sync.dma_start(
                    out=xTt[:, 2 * g:2 * g + 2, :],
                    in_=xbT[2 * g * 128:(2 * g + 2) * 128, :].rearrange(
                        "(c p) t -> p c t", c=2))
            for w_dram, wt in ((wk_s, wts_k), (wv_s, wts_v)):
                for g in range(2):
                    nc.scalar.dma_start(
                        out=wt[:, 4 * g:4 * g + 4, :],
                        in_=w_dram[512 * g:512 * (g + 1), :].rearrange(
                            "(c p) n -> p c n", c=4))
            nc.scalar.dma_start(
                out=wo_sb[:],
                in_=wo_s[:, :].rearrange("(c p) n -> p c n", c=NPAIR))

            # ---- per-head normalization staging ------------------------------
            mst_q = nsml.tile([2, NPAIR * T], HOT, tag="mstq")
            mst_k = nsml.tile([2, NPAIR * T], HOT, tag="mstk")
            rstd_q = mst_q
            rstd_k = mst_k
            rstd_d = drp.tile([2, NPAIR * T], HOT, tag="rstd_d")
            rstd_kd = drp.tile([2, NPAIR * T], HOT, tag="rstd_kd")
            qraw = [qtp.tile([128, T], HOT, tag="qt", name=f"qraw{p}")
                    for p in range(NPAIR)]
            kraw = [qtp.tile([128, T], HOT, tag="qt", name=f"kraw{p}")
                    for p in range(NPAIR)]

            dn_all = [dnp.tile([2, T], F32, tag=f"dn{p}", name=f"dn{p}")
                      for p in range(NPAIR)]
            dnr = [dnp.tile([2, T], F32, tag=f"dnr{p}", name=f"dnr{p}")
                   for p in range(NPAIR)]
            ohp = [ohpool.tile([128, T], HOT, tag=f"ohp{p}", name=f"ohp{p}")
                   for p in range(NPAIR)]
            ohr = [ohpool.tile([128, T], HOT, tag=f"ohr{p}", name=f"ohr{p}")
                   for p in range(NPAIR)]

            # =================================================================
            # thunk machinery: fillers are emitted between attention kt-steps
            # =================================================================
            fillers = deque()

            def pump(n):
                for _ in range(n):
                    if not fillers:
                        return
                    fillers.popleft()()

            def drain():
                while fillers:
                    fillers.popleft()()

            def proj_thunks(wts, pair, kind, on_act):
                """one projection (Q/K/V) for one pair as a list of thunks.
                kind: 'q'|'k'|'v'; on_act: evictions/squares on ACT (True
                during the warm-up window) or DVE (during attention)."""
                thunks = []
                for qh in range(QH):
                    for qq in range(2):
                        box = {}
                        cols = slice(qh * 1024 + qq * 512,
                                     qh * 1024 + (qq + 1) * 512)

                        def mk_mm(ct, cols=cols, box=box):
                            def t():
                                if ct == 0:
                                    box["pj"] = ps_big.tile([128, 512], F32,
                                                            tag="big",
                                                            name="pj")
                                nc.tensor.matmul(
                                    box["pj"][:],
                                    wts[:, ct, pair * 128:(pair + 1) * 128],
                                    xTt[:, ct, cols],
                                    start=(ct == 0), stop=(ct == CT - 1))
                            return t

                        for ct in range(CT):
                            thunks.append(mk_mm(ct))

                        def post(qh=qh, qq=qq, cols=cols, box=box):
                            pj = box["pj"]
                            if kind == "v":
                                dst = vstage[pair]
                                if on_act:
                                    nc.scalar.copy(dst[:, cols], pj[:])
                                else:
                                    with nc.allow_low_precision(reason="bf16"):
                                        nc.vector.tensor_copy(out=dst[:, cols],
                                                              in_=pj[:])
                                return
                            raw = qraw[pair] if kind == "q" else kraw[pair]
                            mst = mst_q if kind == "q" else mst_k
                            q2 = q2p.tile([128, 512], HOT, tag="q2")
                            with nc.allow_low_precision(reason="bf16 raw"):
                                if on_act:
                                    nc.scalar.copy(raw[:, cols], pj[:])
                                    nc.scalar.square(q2[:], pj[:])
                                else:
                                    nc.vector.tensor_copy(out=raw[:, cols],
                                                          in_=pj[:])
                                    nc.vector.tensor_mul(q2[:], raw[:, cols],
                                                         raw[:, cols])
                            ssp = ps_s.tile([2, 512], F32, tag="sbig",
                                            name="ssp")
                            nc.tensor.matmul(ssp[:], bd2[:], q2[:],
                                             start=True, stop=True)
                            with nc.allow_low_precision(reason="bf16 ms"):
                                nc.vector.tensor_copy(
                                    out=mst[0:2,
                                            pair * T + qh * 1024 + qq * 512:
                                            pair * T + qh * 1024 + (qq + 1) * 512],
                                    in_=ssp[:])

                        thunks.append(post)
                return thunks

            vstage = [vtp.tile([128, T], HOT, tag="vt", name=f"vst_{p}")
                      for p in range(NPAIR)]

            def vtrans_thunks(pair, on_act):
                thunks = []
                for tt in range(TT):
                    def t(tt=tt):
                        vb = ps_big.tile([128, 128], HOT, tag="big", name="vb")
                        nc.tensor.transpose(
                            vb[:], vstage[pair][:, tt * 128:(tt + 1) * 128],
                            ident[:])
                        with nc.allow_low_precision(reason="bf16"):
                            if on_act:
                                nc.scalar.copy(
                                    vp[pair][:, tt, :, 0:64],
                                    vb[:].rearrange("p (h d) -> p h d", h=2))
                            else:
                                nc.vector.tensor_copy(
                                    out=vp[pair][:, tt, :, 0:64],
                                    in_=vb[:].rearrange("p (h d) -> p h d", h=2))
                    thunks.append(t)
                return thunks

            def rstd_thunks(pair):
                """ACT: ln+exp for this pair's q and k sumsq; DMA out."""
                pT = pair * T
                def t1():
                    with nc.allow_low_precision(reason="bf16 rstd"):
                        nc.scalar.activation(mst_q[0:2, pT:pT + T],
                                             mst_q[0:2, pT:pT + T], Log,
                                             scale=1.0 / HD, bias=epsc[:])
                def t2():
                    with nc.allow_low_precision(reason="bf16 rstd"):
                        nc.scalar.activation(mst_k[0:2, pT:pT + T],
                                             mst_k[0:2, pT:pT + T], Log,
                                             scale=1.0 / HD, bias=epsc[:])
                def t3():
                    with nc.allow_low_precision(reason="bf16 rstd"):
                        nc.scalar.activation(rstd_q[0:2, pT:pT + T],
                                             mst_q[0:2, pT:pT + T], Exp,
                                             scale=-0.5)
                    nc.sync.dma_start(out=rstd_d[0:2, pT:pT + T],
                                      in_=rstd_q[0:2, pT:pT + T])
                def t4():
                    with nc.allow_low_precision(reason="bf16 rstd"):
                        nc.scalar.activation(rstd_k[0:2, pT:pT + T],
                                             mst_k[0:2, pT:pT + T], Exp,
                                             scale=-0.5)
                    nc.sync.dma_start(out=rstd_kd[0:2, pT:pT + T],
                                      in_=rstd_k[0:2, pT:pT + T])
                return [t1, t2, t3, t4]

            def norm_thunks(pair):
                """broadcast rstd rows and multiply raw q/k into qhat/khat."""
                thunks = []
                for rdram, raw, dest in ((rstd_d, qraw, qhat),
                                         (rstd_kd, kraw, khat)):
                    for qh in range(QH):
                        def t(rdram=rdram, raw=raw, dest=dest, qh=qh):
                            sl = slice(qh * 1024, (qh + 1) * 1024)
                            rwsb = q2p.tile([128, 1024], HOT, tag="rwsb", bufs=2)
                            for i in range(2):
                                row = rdram[i:i + 1, pair * T + qh * 1024:
                                            pair * T + (qh + 1) * 1024]
                                brd = bass.AP(tensor=row.tensor, offset=row.offset,
                                              ap=[[0, 64]] + list(row.ap[1:]))
                                nc.sync.dma_start(out=rwsb[64 * i:64 * i + 64, :],
                                                  in_=brd)
                            with nc.allow_low_precision(reason="bf16"):
                                for i in range(2):
                                    rows = slice(64 * i, 64 * i + 64)
                                    nc.vector.tensor_mul(
                                        dest[pair * 2 + i][rows, sl],
                                        raw[pair][rows, sl],
                                        rwsb[rows, :])
                        thunks.append(t)
                return thunks

            # ---- attention unit ---------------------------------------------
            def attn_unit(h, qh, per_kt_pump):
                pair, i = h // 2, h % 2
                Ks = khat[h]
                Qs = qhat[h]
                o_h = [ps_o.tile([65, 512], F32, tag="o", name=f"o{h}{qh}{qq}")
                       for qq in range(2)]
                s_tiles = [None] * TT
                p_tiles = [None] * TT

                def emit_scores(kt):
                    s_ps = ps_s.tile([128, 1024], F32, tag="sbig")
                    for qq in range(2):
                        nc.tensor.matmul(
                            s_ps[:, qq * 512:(qq + 1) * 512],
                            Ks[:, kt * 128:(kt + 1) * 128],
                            Qs[:, qh * 1024 + qq * 512:qh * 1024 + (qq + 1) * 512],
                            start=True, stop=True)
                    s_tiles[kt] = s_ps

                def emit_exp(kt):
                    p_sb = ppool.tile([128, 1024], HOT, tag="p")
                    nc.scalar.activation(p_sb[:], s_tiles[kt][:], Exp,
                                         scale=0.125)
                    p_tiles[kt] = p_sb

                def emit_av(kt):
                    for qq in range(2):
                        nc.tensor.matmul(
                            o_h[qq][:, :],
                            vp[pair][:, kt, i, :],
                            p_tiles[kt][:, qq * 512:(qq + 1) * 512],
                            start=(kt == 0), stop=(kt == TT - 1))

                emit_scores(0)
                emit_exp(0)
                for kt in range(1, TT):
                    emit_scores(kt)
                    emit_exp(kt)
                    emit_av(kt - 1)
                    pump(per_kt_pump)
                emit_av(TT - 1)

                sl = slice(qh * 1024, (qh + 1) * 1024)
                dnt = dntp.tile([65, 1024], F32, tag="dnt")
                for qq in range(2):
                    nc.vector.tensor_copy(out=dnt[64:65, qq * 512:(qq + 1) * 512],
                                          in_=o_h[qq][64:65, :])
                nc.sync.dma_start(out=dn_all[pair][i:i + 1, sl],
                                  in_=dnt[64:65, :])
                if i == 0:
                    with nc.allow_low_precision(reason="bf16 o"):
                        for qq in range(2):
                            nc.vector.tensor_copy(
                                out=ohp[pair][0:64, qh * 1024 + qq * 512:
                                              qh * 1024 + (qq + 1) * 512],
                                in_=o_h[qq][0:64, :])
                else:
                    obuf = dntp.tile([64, 1024], HOT, tag="ob", name="obuf")
                    with nc.allow_low_precision(reason="bf16 o"):
                        for qq in range(2):
                            nc.vector.tensor_copy(
                                out=obuf[0:64, qq * 512:(qq + 1) * 512],
                                in_=o_h[qq][0:64, :])
                    nc.sync.dma_start(out=ohp[pair][64:128, sl], in_=obuf[0:64, :])
                pump(2)

            # ---- normalization / output projection thunks -------------------
            def normalize_thunks(pair, qhs):
                thunks = []
                c0, c1 = qhs[0] * 1024, (qhs[-1] + 1) * 1024

                def recip():
                    nc.vector.reciprocal_approx_fast(
                        out=dnr[pair][:, c0:c1], in_=dn_all[pair][:, c0:c1])
                thunks.append(recip)
                for qh2 in qhs:
                    for qq in range(2):
                        def t(qh2=qh2, qq=qq):
                            cols = slice(qh2 * 1024 + qq * 512,
                                         qh2 * 1024 + (qq + 1) * 512)
                            rb = ps_big.tile([128, 512], F32, tag="big")
                            nc.tensor.matmul(rb[:], sel[pair][0:2, :],
                                             dnr[pair][0:2, cols],
                                             start=True, stop=True)
                            with nc.allow_low_precision(reason="bf16"):
                                nc.vector.tensor_mul(ohr[pair][:, cols],
                                                     ohp[pair][:, cols], rb[:])
                        thunks.append(t)
                return thunks

            def outproj_thunks(th, ev_act_ok):
                thunks = []
                for et in range(D // 128):
                    box = {}

                    def mk(qq, et=et, box=box):
                        def t():
                            if qq == 0:
                                box["osb"] = outsbp.tile([128, 1024], HOT,
                                                         tag="outsb",
                                                         name="osb")
                            ops = ps_big.tile([128, 512], F32, tag="big")
                            for cp in range(NPAIR):
                                nc.tensor.matmul(
                                    ops[:],
                                    wo_sb[:, cp, et * 128:(et + 1) * 128],
                                    ohr[cp][:, th * 1024 + qq * 512:
                                            th * 1024 + (qq + 1) * 512],
                                    start=(cp == 0), stop=(cp == NPAIR - 1))
                            with nc.allow_low_precision(reason="bf16 out"):
                                if ev_act_ok and qq == 0:
                                    nc.scalar.copy(
                                        box["osb"][:, qq * 512:(qq + 1) * 512],
                                        ops[:])
                                else:
                                    nc.vector.tensor_copy(
                                        out=box["osb"][:, qq * 512:(qq + 1) * 512],
                                        in_=ops[:])
                            if qq == 1:
                                (nc.scalar if (ev_act_ok and et % 2 == 0)
                                 else nc.sync).dma_start(
                                    out=outT[et * 128:(et + 1) * 128,
                                             th * 1024:(th + 1) * 1024],
                                    in_=box["osb"][:])
                        return t

                    thunks.append(mk(0))
                    thunks.append(mk(1))
                return thunks

            # =================================================================
            # schedule
            # =================================================================
            # warm-up window: pair0 projections inline (ACT evictions)
            for t in proj_thunks(wts_q, 0, "q", on_act=True):
                t()
            for t in proj_thunks(wts_k, 0, "k", on_act=True):
                t()
            for t in rstd_thunks(0):
                t()
            for t in proj_thunks(wts_v, 0, "v", on_act=True):
                t()
            for t in vtrans_thunks(0, on_act=True):
                t()
            for t in norm_thunks(0):
                t()

            # pair1 work rides inside pair0's attention
            fillers.extend(proj_thunks(wts_q, 1, "q", on_act=False))
            fillers.extend(proj_thunks(wts_k, 1, "k", on_act=False))
            fillers.extend(rstd_thunks(1))
            fillers.extend(proj_thunks(wts_v, 1, "v", on_act=False))
            fillers.extend(vtrans_thunks(1, on_act=False))
            fillers.extend(norm_thunks(1))

            for qh in range(QH):
                attn_unit(0, qh, 2)
            for qh in range(QH):
                attn_unit(1, qh, 2)
            drain()

            # pair0 normalization rides inside pair1's attention
            fillers.extend(normalize_thunks(0, (0, 1)))
            for qh in range(QH):
                attn_unit(2, qh, 1)
            attn_unit(3, 0, 1)
            drain()
            # first output half rides inside the last attention unit
            fillers.extend(normalize_thunks(1, (0,)))
            fillers.extend(outproj_thunks(0, ev_act_ok=False))
            attn_unit(3, 1, 2)
            drain()

            # tail: the last query half
            for t in normalize_thunks(1, (1,)):
                t()
            for t in outproj_thunks(1, ev_act_ok=True):
                t()

    nc.compile()
    return nc


def _get_compiled():
    global _COMPILED
    if _COMPILED is None:
        _COMPILED = _build()
    return _COMPILED


def _make_consts():
    ident = np.eye(128, dtype=np.float32)
    bd2 = np.zeros((128, 2), np.float32)
    bd2[0:64, 0] = 1.0
    bd2[64:128, 1] = 1.0
    # sel[p] broadcasts dnr[p] row i across partitions 64i:64i+64
    sels = []
    for p in range(2):
        s = np.zeros((128, 128), np.float32)
        s[0, 0:64] = 1.0
        s[1, 64:128] = 1.0
        sels.append(s)
    return ident, bd2, sels


def kernel(x, wq, wk, wv, wo, q_norm_w, k_norm_w):
    from concourse.bass_utils import run_bass_kernel_spmd

    global LAST_EXEC_NS
    if os.environ.get("BASS_TRACE"):
        _install_ntff_shim()

    x = np.asarray(x, dtype=np.float32)
    wq = np.asarray(wq, dtype=np.float32)
    wk = np.asarray(wk, dtype=np.float32)
    wv = np.asarray(wv, dtype=np.float32)
    wo = np.asarray(wo, dtype=np.float32)
    q_norm_w = np.asarray(q_norm_w, dtype=np.float32)
    k_norm_w = np.asarray(k_norm_w, dtype=np.float32)

    nc = _get_compiled()
    ident, bd2, sels = _make_consts()
    import ml_dtypes
    cast = lambda a: a.astype(ml_dtypes.bfloat16)

    in_maps = []
    for c in range(N_CORES):
        b = c // 4
        hs = HPC * (c % 4)
        # head split in reference is strided: head h uses columns d*H + h
        perm = ((hs + np.arange(HPC))[:, None] + H * np.arange(HD)[None, :]).reshape(-1)
        wq_slice = wq[:, perm] * np.tile(q_norm_w, HPC)[None, :]
        wk_slice = wk[:, perm] * np.tile(k_norm_w, HPC)[None, :]
        in_maps.append({
            "xbT": cast(np.ascontiguousarray(x[b].T)),
            "wq_s": cast(np.ascontiguousarray(wq_slice)),
            "wk_s": cast(np.ascontiguousarray(wk_slice)),
            "wv_s": cast(np.ascontiguousarray(wv[:, perm])),
            "wo_s": cast(np.ascontiguousarray(wo[hs * HD:(hs + HPC) * HD, :])),
            "ident": cast(ident), "bd2": cast(bd2),
            "sel0": sels[0], "sel1": sels[1],
        })

    res = run_bass_kernel_spmd(nc, in_maps, core_ids=list(range(N_CORES)),
                               trace=bool(os.environ.get("BASS_TRACE")),
                               tmpdir=os.environ.get("BASS_TRACE_DIR"))
    LAST_EXEC_NS = res.exec_time_ns

    out = np.empty((B, T, D), dtype=np.float32)
    for b in range(B):
        acc = res.results[4 * b]["outT"].astype(np.float32)
        for c in range(4 * b + 1, 4 * b + 4):
            acc = acc + res.results[c]["outT"].astype(np.float32)
        out[b] = acc.T
    return out


# revision 35
# speedup vs baseline: 1.1088x; 1.1088x over previous
"""Multi-head attention (QKV proj + per-head RMSNorm + softmax attention +
output proj) for Trainium2, distributed over 8 NeuronCores.

Sharding: batch (2) x head-groups (4 heads per core).  Each core computes, for
its batch element and its 4 heads: Q^T/K^T projections (transposed layout
[d, T], from a host-pretransposed X^T), per-head RMSNorm, S^T = K^T.T @ Q^T
scores in [key, query] layout, exp with no max subtraction, O^T accumulation
via a fused [V|1] matmul, normalization, and a partial output projection
Out^T = Wo_slice.T @ O^T.  The host sums the 4 partial outputs per batch and
transposes back.

Optimization notes (v2):
- K is never normalized: rstd_k/8 rides into the score exp as a per-partition
  activation scale (scores are [key, query] so rstd_k is per-partition).
  The 1/8 fold happens in log-space via the exp bias of the rstd computation.
- ScalarE (ACT) does exp only during the attention phase; all PSUM evictions
  run on DVE.  Softmax denominators use one DVE reciprocal_approx_fast.
- Attention inner loop is software-pipelined: scores(kt+1) is emitted before
  AV(kt) so the PE never waits on ACT's exp.
- Startup DMAs are split across sync+scalar queues and x is loaded in 16
  chunks so the first projection matmuls start within ~5us.
"""

import os
import sys

for _p in ("/opt/trn_rl_repo",):
    if _p not in sys.path:
        sys.path.insert(0, _p)

import numpy as np

B = 2
T = 2048
D = 1024
H = 16
HD = 64
HPC = 4          # heads per core
N_CORES = 8
EPS = 1e-5

_COMPILED = None
LAST_EXEC_NS = None


def _install_ntff_shim():
    """antenv.axon_hooks is missing in this image; provide it so that
    BASS_TRACE=1 profiling works (mirrors trn_boot's ctypes hook)."""
    import contextlib
    import ctypes
    import types

    if "antenv.axon_hooks" in sys.modules:
        return
    so_path = "/opt/axon/libaxon_pjrt.so"
    if not os.path.exists(so_path):
        return
    lib = ctypes.CDLL(so_path)
    if not hasattr(lib, "axon_start_nrt_profile"):
        return
    lib.axon_start_nrt_profile.argtypes = [ctypes.POINTER(ctypes.c_int64), ctypes.c_size_t]
    lib.axon_start_nrt_profile.restype = ctypes.c_int64
    lib.axon_stop_nrt_profile.argtypes = [ctypes.c_char_p]
    lib.axon_stop_nrt_profile.restype = ctypes.c_int64

    @contextlib.contextmanager
    def _hook(output_dir, device_ids):
        import jax

        jax.devices()
        if device_ids:
            ids = (ctypes.c_int64 * len(device_ids))(*device_ids)
            rc = lib.axon_start_nrt_profile(ids, len(device_ids))
        else:
            rc = lib.axon_start_nrt_profile(None, 0)
        if rc != 0:
            raise RuntimeError(f"axon_start_nrt_profile rc={rc}")
        try:
            yield
        finally:
            n = lib.axon_stop_nrt_profile(str(output_dir).encode())
            print(f"profile: {n} file(s) written to {output_dir}", file=sys.stderr)

    mod = types.ModuleType("antenv.axon_hooks")
    mod._hook = _hook
    mod.get_axon_ntff_profile_hook = lambda: mod._hook
    mod.set_axon_ntff_profile_hook = lambda h: setattr(mod, "_hook", h)
    sys.modules["antenv.axon_hooks"] = mod
    try:
        import antenv

        antenv.axon_hooks = mod
    except ImportError:
        pass


def _build():
    import concourse.bass as bass
    import concourse.tile as tile
    from concourse import bacc, mybir

    F32 = mybir.dt.float32
    F32R = mybir.dt.float32r
    BF16 = mybir.dt.bfloat16
    HOT = BF16
    Exp = mybir.ActivationFunctionType.Exp
    Log = mybir.ActivationFunctionType.Ln if hasattr(
        mybir.ActivationFunctionType, "Ln") else mybir.ActivationFunctionType.Log

    TT = T // 128            # 16 t-tiles
    CT = D // 128            # 8 contraction tiles over model dim
    QH = T // 1024           # 2 query halves
    NPAIR = HPC // 2         # 2 head pairs per core

    nc = bacc.Bacc("TRN2", target_bir_lowering=False, debug=False, num_devices=N_CORES)

    HIN = BF16
    xbT = nc.dram_tensor("xbT", (D, T), HIN, kind="ExternalInput").ap()
    wq_s = nc.dram_tensor("wq_s", (D, HPC * HD), HIN, kind="ExternalInput").ap()
    wk_s = nc.dram_tensor("wk_s", (D, HPC * HD), HIN, kind="ExternalInput").ap()
    wv_s = nc.dram_tensor("wv_s", (D, HPC * HD), HIN, kind="ExternalInput").ap()
    wo_s = nc.dram_tensor("wo_s", (HPC * HD, D), HIN, kind="ExternalInput").ap()
    ident_d = nc.dram_tensor("ident", (128, 128), HIN, kind="ExternalInput").ap()
    bd2_d = nc.dram_tensor("bd2", (128, 2), HIN, kind="ExternalInput").ap()
    sel_d = [nc.dram_tensor(f"sel{p}", (128, 128), F32, kind="ExternalInput").ap()
             for p in range(NPAIR)]
    outT = nc.dram_tensor("outT", (D, T), HIN, kind="ExternalOutput").ap()

    with tile.TileContext(nc) as tc:
        from contextlib import ExitStack

        with ExitStack() as top:
            # ---- persistent pools -------------------------------------------------
            consts = top.enter_context(tc.tile_pool(name="consts", bufs=1))
            qkpool = top.enter_context(tc.tile_pool(name="qk", bufs=1))
            vppool = top.enter_context(tc.tile_pool(name="vp", bufs=1))
            drp = top.enter_context(tc.tile_pool(name="drs", bufs=1, space="DRAM"))

            ident = consts.tile([128, 128], HOT, tag="ident")
            nc.sync.dma_start(out=ident[:], in_=ident_d.bitcast(HOT))
            epsc = consts.tile([2, 1], F32, tag="epsc")
            nc.vector.memset(epsc[:], EPS)
            bd2 = consts.tile([128, 2], HOT, tag="bd2")
            nc.sync.dma_start(out=bd2[:], in_=bd2_d)
            sel = []
            for p in range(NPAIR):
                s = consts.tile([128, 128], F32R, tag=f"sel{p}", name=f"sel{p}")
                nc.sync.dma_start(out=s[:], in_=sel_d[p].bitcast(F32R))
                sel.append(s)

            # persistent data tiles, zero-padded to full 128 contraction rows
            qhat = [qkpool.tile([128, T], HOT, tag=f"qh{h}", name=f"qhat{h}")
                    for h in range(HPC)]
            khat = [qkpool.tile([128, T], HOT, tag=f"kh{h}", name=f"khat{h}")
                    for h in range(HPC)]
            for h in range(HPC):
                nc.gpsimd.memset(qhat[h][:], 0.0)
                nc.gpsimd.memset(khat[h][:], 0.0)
            # V staging: [128 keys, TT, 2, 65]; [:,tt,h,:] = [V_h|1]
            vp = [vppool.tile([128, TT, 2, 65], HOT, tag=f"vs{p}", name=f"vst{p}")
                  for p in range(NPAIR)]
            for p in range(NPAIR):
                nc.vector.memset(vp[p][:, :, :, 64:65], 1.0)

            # =============== Phase 0+1: X^T, projections, RMS norm ================
            with ExitStack() as p01:
                ps_big = p01.enter_context(
                    tc.tile_pool(name="psbig", bufs=3, space="PSUM"))
                ps_sml = p01.enter_context(
                    tc.tile_pool(name="pssml", bufs=2, space="PSUM"))
                xtp = p01.enter_context(tc.tile_pool(name="xT", bufs=1))
                wpool = p01.enter_context(tc.tile_pool(name="w", bufs=1))
                qtp = p01.enter_context(tc.tile_pool(name="qt", bufs=4))
                vtp = p01.enter_context(tc.tile_pool(name="vt", bufs=2))
                q2p = p01.enter_context(tc.tile_pool(name="q2", bufs=2))
                nsml = p01.enter_context(tc.tile_pool(name="nsml", bufs=1))

                # ---- X^T as one [128, CT, T] tile, loaded in 4 big DMAs ----
                # (a single dma_start already stripes over all 16 DMA engines;
                # fewer triggers = less issue serialization)
                xTt = xtp.tile([128, CT, T], HOT, tag="xT", name="xTt")
                # weights as [128, CT, 256] tiles, 2 DMAs each, issued on the
                # scalar queue (ACT is idle at startup)
                wts_q = wpool.tile([128, CT, HPC * HD], HOT, tag="wq", name="wtq")
                wts_k = wpool.tile([128, CT, HPC * HD], HOT, tag="wk", name="wtk")
                wts_v = wpool.tile([128, CT, HPC * HD], HOT, tag="wv", name="wtv")
                nc.scalar.dma_start(
                    out=wts_q[:, 0:4, :],
                    in_=wq_s[0:512, :].rearrange("(c p) n -> p c n", c=4))
                for g in range(4):
                    nc.sync.dma_start(
                        out=xTt[:, 2 * g:2 * g + 2, :],
                        in_=xbT[2 * g * 128:(2 * g + 2) * 128, :].rearrange(
                            "(c p) t -> p c t", c=2))
                nc.scalar.dma_start(
                    out=wts_q[:, 4:8, :],
                    in_=wq_s[512:1024, :].rearrange("(c p) n -> p c n", c=4))
                for w_dram, wt in ((wk_s, wts_k), (wv_s, wts_v)):
                    for g in range(2):
                        nc.scalar.dma_start(
                            out=wt[:, 4 * g:4 * g + 4, :],
                            in_=w_dram[512 * g:512 * (g + 1), :].rearrange(
                                "(c p) n -> p c n", c=4))

                # ---- projections ----
                def project(wts):
                    for pair in range(NPAIR):
                        for qh in range(QH):
                            pj = ps_big.tile([128, 1024], F32, tag="big")
                            for ct in range(CT):
                                for qq in range(2):
                                    nc.tensor.matmul(
                                        pj[:, qq * 512:(qq + 1) * 512],
                                        wts[:, ct, pair * 128:(pair + 1) * 128],
                                        xTt[:, ct, qh * 1024 + qq * 512:
                                            qh * 1024 + (qq + 1) * 512],
                                        start=(ct == 0), stop=(ct == CT - 1))
                            yield pair, qh, pj

                # per-head sumsq rows: [2 heads-in-pair, pair*T + t]
                # (partition base must stay 0 for DVE/PE access rules)
                mst_q = nsml.tile([2, NPAIR * T], F32, tag="mstq")
                mst_k = nsml.tile([2, NPAIR * T], F32, tag="mstk")

                def proj_stage(wts, name, mst, evict):
                    """projection + sumsq into mst rows; evict(pair, qh, pj)"""
                    for pair, qh, pj in project(wts):
                        sl = slice(qh * 1024, (qh + 1) * 1024)
                        evict(pair, qh, sl, pj)
                        q2 = q2p.tile([128, 1024], HOT, tag="q2")
                        with nc.allow_low_precision(reason="bf16 sumsq"):
                            nc.scalar.square(q2[:], pj[:])
                        for qq in range(2):
                            ss = ps_sml.tile([2, 512], F32, tag="sml")
                            nc.tensor.matmul(ss[:], bd2[:],
                                             q2[:, qq * 512:(qq + 1) * 512],
                                             start=True, stop=True)
                            nc.vector.tensor_copy(
                                out=mst[0:2, pair * T + qh * 1024 + qq * 512:
                                        pair * T + qh * 1024 + (qq + 1) * 512],
                                in_=ss[:])

                # Q: keep raw fp32 for the later rstd multiply
                qt_q = {}

                def evict_q(pair, qh, sl, pj):
                    if pair not in qt_q:
                        qt_q[pair] = qtp.tile([128, T], F32, tag="qt",
                                              name=f"qtq{pair}")
                    nc.scalar.copy(qt_q[pair][:, sl], pj[:])

                # K: raw fp32 staging like Q; rstd_k multiply happens below
                kt_q = {}

                def evict_k(pair, qh, sl, pj):
                    if pair not in kt_q:
                        kt_q[pair] = qtp.tile([128, T], F32, tag="qt",
                                              name=f"ktq{pair}")
                    nc.scalar.copy(kt_q[pair][:, sl], pj[:])

                proj_stage(wts_q, "q", mst_q, evict_q)
                proj_stage(wts_k, "k", mst_k, evict_k)

                # ---- rstd_q (plain) and rstd_k (folded *1/8 via exp bias) ----
                # rstd = (ms/64+eps)^-1/2 = exp(-0.5*ln(ms/64+eps) [+ ln(1/8)])
                # (ACT work; PE continues with the V projection below)
                import math
                nc.scalar.activation(mst_q[:], mst_q[:], Log, scale=1.0 / HD,
                                     bias=epsc[:])
                nc.scalar.activation(mst_k[:], mst_k[:], Log, scale=1.0 / HD,
                                     bias=epsc[:])
                rstd_q = nsml.tile([2, NPAIR * T], F32, tag="rstd_q")
                rstd_k = nsml.tile([2, NPAIR * T], F32, tag="rstd_k")
                nc.scalar.activation(rstd_q[:], mst_q[:], Exp, scale=-0.5)
                nc.scalar.activation(rstd_k[:], mst_k[:], Exp, scale=-0.5)
                rstd_d = drp.tile([2, NPAIR * T], F32, tag="rstd_d")
                rstd_kd = drp.tile([2, NPAIR * T], F32, tag="rstd_kd")
                nc.sync.dma_start(out=rstd_d[:], in_=rstd_q[:])
                nc.scalar.dma_start(out=rstd_kd[:], in_=rstd_k[:])

                # q/k: partition-broadcast the [1, t] rstd rows via DMA, multiply
                def norm_mult(rdram, raw, dest, trig):
                    for pair in range(NPAIR):
                        for qh in range(QH):
                            sl = slice(qh * 1024, (qh + 1) * 1024)
                            rwsb = q2p.tile([128, 1024], F32, tag="rwsb", bufs=2)
                            for i in range(2):
                                row = rdram[i:i + 1, pair * T + qh * 1024:
                                            pair * T + (qh + 1) * 1024]
                                brd = bass.AP(tensor=row.tensor, offset=row.offset,
                                              ap=[[0, 64]] + list(row.ap[1:]))
                                trig.dma_start(out=rwsb[64 * i:64 * i + 64, :],
                                               in_=brd)
                            with nc.allow_low_precision(reason="fp32r rounding"):
                                for i in range(2):
                                    rows = slice(64 * i, 64 * i + 64)
                                    nc.vector.tensor_mul(
                                        dest[pair * 2 + i][rows, sl],
                                        raw[pair][rows, sl],
                                        rwsb[rows, :])

                norm_mult(rstd_d, qt_q, qhat, nc.sync)
                norm_mult(rstd_kd, kt_q, khat, nc.sync)

                # ---- V: project to V^T then transpose into [V|1] tiles ----
                for pair, qh, pj in project(wts_v):
                    if qh == 0:
                        vt_sb = vtp.tile([128, T], HOT, tag="vt")
                        vt_cur = vt_sb
                    else:
                        vt_sb = vt_cur
                    nc.scalar.copy(vt_sb[:, qh * 1024:(qh + 1) * 1024], pj[:])
                    if qh == QH - 1:
                        for tt in range(TT):
                            # vb tiles live in the small-psum pool so the pj
                            # pipeline in ps_big is never blocked
                            vb_ps = ps_sml.tile([128, 128], HOT, tag="sml",
                                                name="vb_ps")
                            nc.tensor.transpose(
                                vb_ps[:], vt_sb[:, tt * 128:(tt + 1) * 128],
                                ident[:])
                            with nc.allow_low_precision(reason="bf16"):
                                nc.scalar.copy(
                                    vp[pair][:, tt, :, 0:64],
                                    vb_ps[:].rearrange("p (h d) -> p h d", h=2))

            # scheduler fence: nothing from P2/P3 may be hoisted before P0/P1
            tc.no_sync_barrier()

            # =============== Phase 2+3: attention + output projection =============
            with ExitStack() as p23:
                ppool = p23.enter_context(tc.tile_pool(name="p", bufs=4))
                dntp = p23.enter_context(tc.tile_pool(name="dnt", bufs=3))
                ps_sbig = p23.enter_context(
                    tc.tile_pool(name="pssbig", bufs=2, space="PSUM"))
                ps_rb = p23.enter_context(
                    tc.tile_pool(name="psrb", bufs=1, space="PSUM"))
                ps_o = p23.enter_context(
                    tc.tile_pool(name="pso", bufs=1, space="PSUM"))
                dnp = p23.enter_context(tc.tile_pool(name="dn", bufs=1))
                ohpool = p23.enter_context(tc.tile_pool(name="ohp", bufs=1))
                outsbp = p23.enter_context(tc.tile_pool(name="outsb", bufs=3))
                wop = p23.enter_context(tc.tile_pool(name="wo", bufs=2))

                dn_all = [dnp.tile([2, T], F32, tag=f"dn{p}", name=f"dn{p}")
                          for p in range(NPAIR)]
                wo_sb = wop.tile([128, NPAIR, D], HOT, tag="wo", name="wot")
                nc.scalar.dma_start(
                    out=wo_sb[:],
                    in_=wo_s[:, :].rearrange("(c p) n -> p c n", c=NPAIR))
                ohp = [ohpool.tile([128, T], F32, tag=f"ohp{p}", name=f"ohp{p}")
                       for p in range(NPAIR)]
                ohr = [ohpool.tile([128, T], HOT, tag=f"ohr{p}", name=f"ohr{p}")
                       for p in range(NPAIR)]
                # per-pair reciprocal staging (pair0 computed mid-attention)
                dnr = [dnp.tile([128, T], F32R, tag=f"dnr{p}", name=f"dnr{p}")
                       for p in range(NPAIR)]
                dnf = dnp.tile([2, T], F32, tag="dnf", name="dnf")
                for p in range(NPAIR):
                    nc.gpsimd.memset(dnr[p][:, :].bitcast(F32), 0.0)

                def normalize(pair, qhs):
                    """1/dn -> dnr rows, rb broadcast matmul, ohr multiply,
                    restricted to the given qh halves."""
                    c0, c1 = qhs[0] * 1024, (qhs[-1] + 1) * 1024
                    nc.vector.reciprocal_approx_fast(
                        out=dnf[:, c0:c1], in_=dn_all[pair][:, c0:c1])
                    with nc.allow_low_precision(reason="fp32r rounding"):
                        nc.vector.tensor_copy(out=dnr[pair][0:2, c0:c1],
                                              in_=dnf[:, c0:c1])
                    for qh2 in qhs:
                        sl2 = slice(qh2 * 1024, (qh2 + 1) * 1024)
                        rb = ps_rb.tile([128, 1024], F32, tag="rb")
                        for qq in range(2):
                            nc.tensor.matmul(
                                rb[:, qq * 512:(qq + 1) * 512], sel[pair][:],
                                dnr[pair][:, qh2 * 1024 + qq * 512:
                                          qh2 * 1024 + (qq + 1) * 512],
                                start=True, stop=True)
                        with nc.allow_low_precision(reason="fp32r rounding"):
                            nc.vector.tensor_mul(ohr[pair][:, sl2],
                                                 ohp[pair][:, sl2], rb[:])

                def outproj(th, ev_act_ok):
                    """output projection for one query half th."""
                    for et in range(D // 128):
                        osb = outsbp.tile([128, 1024], HOT, tag="outsb",
                                          name=f"osb{et}_{th}")
                        ops = ps_rb.tile([128, 1024], F32, tag="rb")
                        for cp in range(NPAIR):
                            for qq in range(2):
                                nc.tensor.matmul(
                                    ops[:, qq * 512:(qq + 1) * 512],
                                    wo_sb[:, cp, et * 128:(et + 1) * 128],
                                    ohr[cp][:, th * 1024 + qq * 512:
                                            th * 1024 + (qq + 1) * 512],
                                    start=(cp == 0), stop=(cp == NPAIR - 1))
                        with nc.allow_low_precision(reason="bf16 partial output"):
                            if ev_act_ok and et % 2 == 0:
                                nc.scalar.copy(osb[:], ops[:])
                            else:
                                nc.vector.tensor_copy(out=osb[:], in_=ops[:])
                        (nc.scalar if (ev_act_ok and et % 2 == 0)
                         else nc.sync).dma_start(
                            out=outT[et * 128:(et + 1) * 128,
                                     th * 1024:(th + 1) * 1024],
                            in_=osb[:])

                # attention: pair0 heads first; pair0 normalization overlaps
                # pair1 attention, and the first output-projection half runs
                # under the last attention unit.
                for h in range(HPC):
                    pair, i = h // 2, h % 2
                    Ks = khat[h]
                    Qs = qhat[h]
                    for qh in range(QH):
                        o_ps = ps_o.tile([128, 1024], F32, tag="o")
                        orows = slice(0, 65)
                        vcol = i
                        # software pipeline: scores(kt) run one step ahead of
                        # AV(kt-1) so the PE never waits on ACT's exp.
                        s_tiles = [None] * TT
                        p_tiles = [None] * TT

                        def emit_scores(kt):
                            s_ps = ps_sbig.tile([128, 1024], F32, tag="sbig")
                            for qq in range(2):
                                nc.tensor.matmul(
                                    s_ps[:, qq * 512:(qq + 1) * 512],
                                    Ks[:, kt * 128:(kt + 1) * 128],
                                    Qs[:, qh * 1024 + qq * 512:qh * 1024 + (qq + 1) * 512],
                                    start=True, stop=True)
                            s_tiles[kt] = s_ps

                        def emit_exp(kt):
                            p_sb = ppool.tile([128, 1024], HOT, tag="p")
                            nc.scalar.activation(p_sb[:], s_tiles[kt][:], Exp,
                                                 scale=0.125)
                            p_tiles[kt] = p_sb

                        def emit_av(kt):
                            for qq in range(2):
                                nc.tensor.matmul(
                                    o_ps[orows, qq * 512:(qq + 1) * 512],
                                    vp[pair][:, kt, vcol, :],
                                    p_tiles[kt][:, qq * 512:(qq + 1) * 512],
                                    start=(kt == 0), stop=(kt == TT - 1))

                        emit_scores(0)
                        emit_exp(0)
                        for kt in range(1, TT):
                            emit_scores(kt)
                            emit_exp(kt)
                            emit_av(kt - 1)
                        emit_av(TT - 1)

                        # evict raw O^T rows + denominator row on DVE
                        sl = slice(qh * 1024, (qh + 1) * 1024)
                        ev = lambda o_, i_: nc.vector.tensor_copy(out=o_, in_=i_)
                        dnt = dntp.tile([65, 1024], F32, tag="dnt")
                        ev(dnt[64:65, :], o_ps[64:65, :])
                        nc.sync.dma_start(out=dn_all[pair][i:i + 1, sl],
                                          in_=dnt[64:65, :])
                        if i == 0:
                            ev(ohp[pair][0:64, sl], o_ps[0:64, :])
                        else:
                            # cross-partition move: tmp rows 0:64, DMA remap
                            ev(dnt[0:64, :], o_ps[0:64, :])
                            nc.sync.dma_start(out=ohp[pair][64:128, sl],
                                              in_=dnt[0:64, :])

                        if h == 1 and qh == QH - 1:
                            # pair0 done: normalize it under pair1's attention
                            normalize(0, (0, 1))
                        if h == HPC - 1 and qh == 0:
                            # qh0 denominators known: normalize under the last
                            # attention unit
                            normalize(1, (0,))

                # tail: first output half runs while the last reciprocal
                # chain completes on DVE, then the second half
                outproj(0, ev_act_ok=False)
                normalize(1, (1,))
                outproj(1, ev_act_ok=True)

    nc.compile()
    return nc


def _get_compiled():
    global _COMPILED
    if _COMPILED is None:
        _COMPILED = _build()
    return _COMPILED


def _make_consts():
    ident = np.eye(128, dtype=np.float32)
    bd2 = np.zeros((128, 2), np.float32)
    bd2[0:64, 0] = 1.0
    bd2[64:128, 1] = 1.0
    # sel[p] broadcasts dnr[p] row i across partitions 64i:64i+64
    sels = []
    for p in range(2):
        s = np.zeros((128, 128), np.float32)
        s[0, 0:64] = 1.0
        s[1, 64:128] = 1.0
        sels.append(s)
    return ident, bd2, sels


def kernel(x, wq, wk, wv, wo, q_norm_w, k_norm_w):
    from concourse.bass_utils import run_bass_kernel_spmd

    global LAST_EXEC_NS
    if os.environ.get("BASS_TRACE"):
        _install_ntff_shim()

    x = np.asarray(x, dtype=np.float32)
    wq = np.asarray(wq, dtype=np.float32)
    wk = np.asarray(wk, dtype=np.float32)
    wv = np.asarray(wv, dtype=np.float32)
    wo = np.asarray(wo, dtype=np.float32)
    q_norm_w = np.asarray(q_norm_w, dtype=np.float32)
    k_norm_w = np.asarray(k_norm_w, dtype=np.float32)

    nc = _get_compiled()
    ident, bd2, sels = _make_consts()
    import ml_dtypes
    cast = lambda a: a.astype(ml_dtypes.bfloat16)

    in_maps = []
    for c in range(N_CORES):
        b = c // 4
        hs = HPC * (c % 4)
        # head split in reference is strided: head h uses columns d*H + h
        perm = ((hs + np.arange(HPC))[:, None] + H * np.arange(HD)[None, :]).reshape(-1)
        wq_slice = wq[:, perm] * np.tile(q_norm_w, HPC)[None, :]
        wk_slice = wk[:, perm] * np.tile(k_norm_w, HPC)[None, :]
        in_maps.append({
            "xbT": cast(np.ascontiguousarray(x[b].T)),
            "wq_s": cast(np.ascontiguousarray(wq_slice)),
            "wk_s": cast(np.ascontiguousarray(wk_slice)),
            "wv_s": cast(np.ascontiguousarray(wv[:, perm])),
            "wo_s": cast(np.ascontiguousarray(wo[hs * HD:(hs + HPC) * HD, :])),
            "ident": cast(ident), "bd2": cast(bd2),
            "sel0": sels[0], "sel1": sels[1],
        })

    res = run_bass_kernel_spmd(nc, in_maps, core_ids=list(range(N_CORES)),
                               trace=bool(os.environ.get("BASS_TRACE")),
                               tmpdir=os.environ.get("BASS_TRACE_DIR"))
    LAST_EXEC_NS = res.exec_time_ns

    out = np.empty((B, T, D), dtype=np.float32)
    for b in range(B):
        acc = res.results[4 * b]["outT"].astype(np.float32)
        for c in range(4 * b + 1, 4 * b + 4):
            acc = acc + res.results[c]["outT"].astype(np.float32)
        out[b] = acc.T
    return out
